# revision 25
# baseline (speedup 1.0000x reference)
"""Multi-head self-attention (B=2, S=2048, D=1024, H=16) on 8 TRN2 NeuronCores.

Tensor-parallel over heads: each core owns 2 heads. Accepts FULL inputs,
returns FULL output. Host pre-transposes x and slices per-head weights;
each core computes qkv -> per-head LayerNorm -> attention -> partial
output projection (over its 128 embed dims); host sums the 8 partials
and adds the projection bias.

v2 layout of the per-core program:
  passA: per 128-token block: qkv matmuls -> grouped bn_stats -> raw
         q/k + v evicted to SBUF (bf16).
  passB: LN constants for all 32 blocks batched in ~8 wide vector ops
         (combine even/odd bn_stats pipes, one ACT sqrt, one DVE
         approx-reciprocal).
  passC: per block: 4 fused tensor_scalar LN applies (bf16 4x mode),
         2 PE transposes, one copy into the [dim, token] q/k store.
  phase2: software-pipelined attention: scores run 2 k-blocks ahead of
         the exp stream, A@V runs 2 behind, the softmax denominator is
         inverted with a DVE approx reciprocal (no ACT table switches
         ever - ACT does only the exp stream), and each chunk's
         projection is deferred one chunk to fill the PE bubble at the
         chunk boundary.
"""

import os
import sys

import numpy as np

for _p in ("/opt/trn_rl_repo", "/root/.axon_site/_ro/trn_rl_repo"):
    if os.path.isdir(_p) and _p not in sys.path:
        sys.path.insert(0, _p)
        break

import concourse.bass as bass  # noqa: E402
import concourse.bacc as bacc  # noqa: E402
import concourse.tile as tile  # noqa: E402
from concourse import mybir  # noqa: E402
from concourse.bass_utils import run_bass_kernel_spmd  # noqa: E402

F32 = mybir.dt.float32
F32R = mybir.dt.float32r
BF16 = mybir.dt.bfloat16
AF = mybir.ActivationFunctionType
ALU = mybir.AluOpType

NCORES = 8
D = 1024
H = 16
HD = 64
HPC = H // NCORES          # heads per core = 2
DPC = HPC * HD             # embed dims per core = 128
KCH_H = D // 128           # contraction chunks (host-side constant)
EPS = 1e-5


def _r(ap):
    return ap.bitcast(F32R)


def build_nc(B, S, affine):
    """Build the SPMD Bass program for one core (same program, 8 cores)."""
    T = B * S                      # total token columns
    NTB = T // 128                 # 128-token blocks (32)
    NCH = T // 512                 # 512-token chunks (8)
    QC = S // 512                  # q-chunks per batch (4)
    KB = S // 128                  # k-blocks per batch (16)
    KCH = D // 128                 # contraction chunks (8)
    SCALE = 1.0 / np.sqrt(HD)

    nc = bacc.Bacc(
        "TRN2",
        target_bir_lowering=False,
        debug=False,
        enable_asserts=True,
        num_devices=NCORES,
    )

    xT = nc.dram_tensor("xT", [D, T], BF16, kind="ExternalInput").ap()
    wq = nc.dram_tensor(
        "wt_qkv", [128, KCH, 3 * DPC], BF16, kind="ExternalInput"
    ).ap()
    bq = nc.dram_tensor("b_qkv_s", [1, 3 * DPC], BF16, kind="ExternalInput").ap()
    wp = nc.dram_tensor("wt_proj", [DPC, D], BF16, kind="ExternalInput").ap()
    ones = nc.dram_tensor("c_ones", [1, 512], F32R, kind="ExternalInput").ap()
    vones = nc.dram_tensor(
        "c_vones", [128, HPC, NTB, 1], BF16, kind="ExternalInput"
    ).ap()
    onesb = nc.dram_tensor("c_onesb", [1, 512], BF16, kind="ExternalInput").ap()
    e2 = nc.dram_tensor("c_e2", [64, 128], F32R, kind="ExternalInput").ap()
    den0 = nc.dram_tensor("c_den0", [64, 512], F32R, kind="ExternalInput").ap()
    eye = nc.dram_tensor("c_eye", [128, 128], BF16, kind="ExternalInput").ap()
    if affine:
        gb = nc.dram_tensor("c_gb", [128, 4, HD], F32, kind="ExternalInput").ap()
    outp = nc.dram_tensor("outp", [T, D], BF16, kind="ExternalOutput").ap()

    from contextlib import ExitStack

    with tile.TileContext(nc) as tc, ExitStack() as stack:
        const = stack.enter_context(tc.tile_pool(name="const", bufs=1))
        persist = stack.enter_context(tc.tile_pool(name="persist", bufs=1))

        # weights needed by passA go out first on the sync queue; the
        # late-use constants ride the gpsimd (SWDGE) queue so the first
        # x chunk isn't stuck behind them.
        bq_sb = const.tile([1, 3 * DPC], BF16, tag="bq")
        nc.sync.dma_start(out=bq_sb, in_=bq)
        onesb_sb = const.tile([1, 512], BF16, tag="onesb")
        nc.sync.dma_start(out=onesb_sb, in_=onesb)
        wq_sb = const.tile([128, KCH, 3 * DPC], BF16, tag="wq")
        nc.sync.dma_start(out=wq_sb, in_=wq)
        eye_sb = const.tile([128, 128], BF16, tag="eye")
        nc.gpsimd.dma_start(out=eye_sb, in_=eye)
        ones_sb = const.tile([1, 512], F32R, tag="ones")
        nc.gpsimd.dma_start(out=ones_sb, in_=ones)
        e2_sb = const.tile([64, 128], F32R, tag="e2")
        nc.gpsimd.dma_start(out=e2_sb, in_=e2)
        wp_sb = const.tile([DPC, D], BF16, tag="wp")
        nc.gpsimd.dma_start(out=wp_sb, in_=wp)
        eps_sb = const.tile([128, 1], F32, tag="eps")
        nc.vector.memset(eps_sb, EPS)
        if affine:
            gb_sb = const.tile([128, 4, HD], F32, tag="gb")
            nc.gpsimd.dma_start(out=gb_sb, in_=gb)

        # persistent intermediates
        qkT = persist.tile([128, 2, T], BF16, tag="qkT")   # plane0=q^T plane1=k^T
        vO = persist.tile([128, HPC, NTB, HD + 1], BF16, tag="vO")
        aT = persist.tile([128, T], BF16, tag="aT")        # attention out^T
        den64 = [
            persist.tile([64, 512], F32R, tag=f"den64{i}", name=f"den64{i}")
            for i in range(2)
        ]
        nc.gpsimd.dma_start(out=den64[0], in_=den0)
        nc.gpsimd.dma_start(out=den64[1], in_=den0)
        nc.gpsimd.dma_start(out=vO[:, :, :, HD : HD + 1], in_=vones)

        # ------- Phase 1: per-block qkv + LN + transpose pipeline --------
        # Work spread over three engines so no single one binds:
        #   PE:  bias+qkv matmuls, 2 transposes
        #   DVE: bn_stats/aggr, rstd recip, -mu*rstd
        #   ACT: sqrt, the 4 LN applies (scale/bias APs), v cast, qkT evict
        with (
            tc.tile_pool(name="xt", bufs=2) as xt_pool,
            tc.tile_pool(name="qkv_ps", bufs=4, space="PSUM") as qkv_ps,
            tc.tile_pool(name="stats", bufs=4) as stats_pool,
            tc.tile_pool(name="qn", bufs=3) as qn_pool,
            tc.tile_pool(name="t_ps", bufs=3, space="PSUM") as t_ps,
        ):
            xTr = xT.rearrange("(c p) t -> p c t", p=128)
            xt0s = []
            for q in range(4):
                x0 = xt_pool.tile(
                    [128, KCH, 128], BF16, tag=f"xt0_{q}", name=f"xt0_{q}"
                )
                nc.sync.dma_start(
                    out=x0, in_=xTr[:, :, q * 128 : (q + 1) * 128]
                )
                xt0s.append(x0)
            for n in range(NCH):
                if n > 0:
                    xt = xt_pool.tile([128, KCH, 512], BF16, tag="xt")
                    nc.sync.dma_start(
                        out=xt, in_=xTr[:, :, n * 512 : (n + 1) * 512]
                    )
                for tbl in range(4):
                    tb = n * 4 + tbl
                    xts = xt0s[tbl] if n == 0 else xt[
                        :, :, tbl * 128 : (tbl + 1) * 128
                    ]
                    ps = qkv_ps.tile([128, 3 * DPC], F32, tag="ps")
                    nc.tensor.matmul(
                        ps,
                        lhsT=onesb_sb[0:1, 0:128],
                        rhs=bq_sb,
                        start=True,
                        stop=False,
                    )
                    for k in range(KCH):
                        nc.tensor.matmul(
                            ps,
                            lhsT=xts[:, k, :],
                            rhs=wq_sb[:, k, :],
                            start=False,
                            stop=(k == KCH - 1),
                        )
                    # host interleaves the two heads' channels in the q
                    # and k column blocks, so one bn_stats per block gives
                    # per-head stats via its even/odd split pipes:
                    # st = [cnt_e, mean_h0, M2_h0, cnt_o, mean_h1, M2_h1]
                    st = stats_pool.tile([128, 2, 6], F32, tag="st")
                    nc.vector.bn_stats(out=st[:, 0], in_=ps[:, 0:DPC])
                    nc.vector.bn_stats(out=st[:, 1], in_=ps[:, DPC : 2 * DPC])
                    stf = st.rearrange("p a (b t) -> p (a b) t", t=3)
                    sd = stats_pool.tile([128, 4], F32, tag="sd")
                    nc.scalar.activation(
                        out=sd, in_=stf[:, :, 2], func=AF.Sqrt,
                        bias=eps_sb, scale=1.0 / HD,
                    )
                    rstd = stats_pool.tile([128, 4], F32, tag="rstd")
                    nc.vector.reciprocal(out=rstd, in_=sd)
                    negmu = stats_pool.tile([128, 4], F32, tag="negmu")
                    nc.vector.tensor_scalar(
                        out=negmu, in0=stf[:, :, 1], scalar1=-1.0,
                        scalar2=None, op0=ALU.mult,
                    )
                    nmr = stats_pool.tile([128, 4], F32, tag="nmr")
                    nc.vector.tensor_mul(nmr, negmu, rstd)
                    qn = qn_pool.tile([128, 2, 2 * HD], BF16, tag="qn")
                    for g in range(4):
                        pl, hh = g // 2, g % 2
                        dsl = qn[:, pl, hh * HD : (hh + 1) * HD]
                        src_g = ps[:, pl * DPC : (pl + 1) * DPC].rearrange(
                            "p (c two) -> p two c", two=2
                        )[:, hh]
                        if pl == 0:
                            # q applies on ACT (idle in phase 1)
                            nc.scalar.activation(
                                out=dsl,
                                in_=src_g,
                                func=AF.Identity,
                                bias=nmr[:, g : g + 1],
                                scale=rstd[:, g : g + 1],
                            )
                        else:
                            # k applies on DVE
                            nc.vector.tensor_scalar(
                                out=dsl,
                                in0=src_g,
                                scalar1=stf[:, g : g + 1, 1],
                                scalar2=rstd[:, g : g + 1],
                                op0=ALU.subtract,
                                op1=ALU.mult,
                            )
                        if affine:
                            nc.vector.tensor_mul(dsl, dsl, gb_sb[:, 2 * pl, :])
                            nc.vector.tensor_add(dsl, dsl, gb_sb[:, 2 * pl + 1, :])
                    nc.scalar.copy(
                        out=vO[:, :, tb, 0:HD],
                        in_=ps[:, 2 * DPC :].rearrange("p (h d) -> p h d", d=HD),
                    )
                    ts_ = slice(tb * 128, (tb + 1) * 128)
                    tp = t_ps.tile([128, 256], BF16, tag="tp")
                    nc.tensor.transpose(tp[:, 0:128], qn[:, 0, :], eye_sb)
                    nc.tensor.transpose(tp[:, 128:256], qn[:, 1, :], eye_sb)
                    nc.scalar.copy(
                        out=qkT[:, :, ts_],
                        in_=tp.rearrange("p (c d) -> p c d", d=128),
                    )

        # ---------------- Phase 2: attention -----------------------------
        with (
            tc.tile_pool(name="sc_ps", bufs=2, space="PSUM") as sc_ps,
            tc.tile_pool(name="o_ps", bufs=1, space="PSUM") as o_ps,
            tc.tile_pool(name="sm_ps", bufs=2, space="PSUM") as sm_ps,
            tc.tile_pool(name="exps", bufs=4) as exps,
            tc.tile_pool(name="stage2", bufs=2) as stage2,
            tc.tile_pool(name="ostage", bufs=2) as ostage,
        ):
            seq = [(ci, kb) for ci in range(B * QC) for kb in range(KB)]
            ooms = {}
            exts = {}

            def emit_scores_exp(ci, kb):
                b, qc = divmod(ci, QC)
                cols = slice(b * S + qc * 512, b * S + (qc + 1) * 512)
                gkb = b * KB + kb
                ks = slice(gkb * 128, (gkb + 1) * 128)
                scp = sc_ps.tile([128, HPC, 512], F32, tag="s", name="scp")
                for h in range(HPC):
                    hp = slice(h * HD, (h + 1) * HD)
                    nc.tensor.matmul(
                        scp[:, h, :],
                        lhsT=qkT[hp, 1, ks],
                        rhs=qkT[hp, 0, cols],
                        start=True,
                        stop=True,
                    )
                ex = exps.tile([128, HPC, 512], BF16, tag="ex", name="ex")
                nc.scalar.activation(out=ex, in_=scp, func=AF.Exp, scale=SCALE)
                exts[(ci, kb)] = ex

            def emit_av(ci, kb):
                b, _ = divmod(ci, QC)
                gkb = b * KB + kb
                if kb == 0:
                    ooms[ci] = o_ps.tile(
                        [HD + 1, HPC, 512], F32, tag="o", name="oom"
                    )
                oom = ooms[ci]
                ex = exts.pop((ci, kb))
                for h in range(HPC):
                    nc.tensor.matmul(
                        oom[:, h, :],
                        lhsT=vO[:, h, gkb, :],
                        rhs=ex[:, h, :],
                        start=(kb == 0),
                        stop=(kb == KB - 1),
                    )

            def emit_tail(ci):
                b, qc = divmod(ci, QC)
                cols = slice(b * S + qc * 512, b * S + (qc + 1) * 512)
                oom = ooms.pop(ci)
                dn = den64[ci % 2]
                nc.vector.tensor_copy(
                    out=dn[0:1, :], in_=oom[HD : HD + 1, 0, :]
                )
                nc.scalar.copy(out=dn[32:33, :], in_=oom[HD : HD + 1, 1, :])
                rb = sm_ps.tile([128, 512], F32, tag="sm", name="rb")
                nc.tensor.matmul(rb, lhsT=e2_sb, rhs=dn, start=True, stop=True)
                rbs = stage2.tile([128, 512], F32, tag="rbs", name="rbs")
                nc.vector.reciprocal_approx_fast(out=rbs, in_=rb)
                for h in range(HPC):
                    nc.vector.tensor_mul(
                        aT[h * HD : (h + 1) * HD, cols],
                        oom[0:HD, h, :],
                        rbs[h * HD : (h + 1) * HD, :],
                    )

            def emit_proj(ci, tbls, alt=False):
                for tbl in tbls:
                    tb = ci * 4 + tbl
                    ob = ostage.tile([128, D], BF16, tag="ob")
                    for nn in range(D // 512):
                        pps = sm_ps.tile([128, 512], F32, tag="sm", name="pps")
                        nc.tensor.matmul(
                            pps,
                            lhsT=aT[:, tb * 128 : (tb + 1) * 128],
                            rhs=wp_sb[:, nn * 512 : (nn + 1) * 512],
                            start=True,
                            stop=True,
                        )
                        osl = ob[:, nn * 512 : (nn + 1) * 512]
                        if alt and (tbl + nn) % 2:
                            # in the post-exp drain ACT is free: share casts
                            nc.scalar.copy(out=osl, in_=pps)
                        else:
                            nc.vector.tensor_copy(out=osl, in_=pps)
                    nc.gpsimd.dma_start(
                        out=outp[tb * 128 : (tb + 1) * 128, :], in_=ob
                    )

            last = B * QC - 1
            for idx, (ci, kb) in enumerate(seq):
                emit_scores_exp(ci, kb)
                if idx >= 2:
                    emit_av(*seq[idx - 2])
                if kb == 1 and ci >= 1:
                    # proj of chunk ci-2 (data long ready) split around the
                    # tail of ci-1 so its matmuls cover the PE bubble while
                    # the DVE runs the denominator-reciprocal chain, but its
                    # PSUM casts don't delay that chain.
                    if ci >= 2:
                        emit_proj(ci - 2, (0, 1))
                    emit_tail(ci - 1)
                elif kb == 3 and ci >= 2:
                    emit_proj(ci - 2, (2, 3))
                elif ci == last and kb == 8:
                    # pull the second-to-last chunk's proj into this chunk
                    # so only proj(last) remains after the exp stream ends
                    emit_proj(last - 1, (0, 1))
                elif ci == last and kb == 12:
                    emit_proj(last - 1, (2, 3))
            emit_av(*seq[-2])
            emit_av(*seq[-1])
            # harmless matmuls keep the PE's HAM clock warm through the
            # serial reciprocal/projection drain (idle >3.4us would halve
            # the PE clock for the rest of the kernel)
            dmy = sc_ps.tile([128, HPC, 512], F32, tag="s", name="dmy")
            for i in range(6):
                nc.tensor.matmul(
                    dmy[:, i % 2, :],
                    lhsT=e2_sb,
                    rhs=den64[(last + 1) % 2],
                    start=True,
                    stop=True,
                )
            emit_tail(last)
            emit_proj(last, (0, 1, 2, 3), alt=True)

    nc.compile()
    return nc


def make_in_maps(x, w_qkv, b_qkv, w_proj, q_gamma, q_beta, k_gamma, k_beta,
                 affine):
    import ml_dtypes

    bf = ml_dtypes.bfloat16
    B, S, _ = x.shape
    T = B * S
    xT = np.ascontiguousarray(x.reshape(T, D).T).astype(bf)
    ones = np.ones((1, 512), np.float32)
    onesb = np.ones((1, 512), bf)
    vones = np.ones((128, HPC, (T // 128), 1), bf)
    eye = np.eye(128, dtype=np.float32).astype(bf)
    in_maps = []
    for c in range(NCORES):
        rs = slice(c * DPC, (c + 1) * DPC)
        w_slice = np.concatenate(
            [w_qkv[rs], w_qkv[D:2 * D][rs.start:rs.stop],
             w_qkv[2 * D:][rs.start:rs.stop]],
            axis=0,
        )  # [384, 1024]
        b_slice = np.concatenate(
            [b_qkv[rs], b_qkv[D:2 * D][rs.start:rs.stop],
             b_qkv[2 * D:][rs.start:rs.stop]]
        )[None, :]  # [1, 384]
        # interleave the two heads' channels inside the q and the k blocks
        # (h0 -> even, h1 -> odd) so bn_stats' even/odd pipes separate them
        perm = np.arange(3 * DPC)
        for blk in range(2):
            base = blk * DPC
            iv = np.empty(DPC, np.int64)
            iv[0::2] = base + np.arange(HD)
            iv[1::2] = base + HD + np.arange(HD)
            perm[base : base + DPC] = iv
        w_slice = w_slice[perm]
        b_slice = b_slice[:, perm]
        wq_l = np.ascontiguousarray(
            w_slice.T.reshape(KCH_H, 128, 3 * DPC).transpose(1, 0, 2)
        ).astype(bf)
        e2 = np.zeros((64, 128), np.float32)
        e2[0, 0:HD] = 1.0
        e2[32, HD:128] = 1.0
        m = {
            "xT": xT,
            "wt_qkv": wq_l,
            "b_qkv_s": np.ascontiguousarray(b_slice).astype(bf),
            "wt_proj": np.ascontiguousarray(w_proj[:, rs].T).astype(bf),
            "c_ones": ones,
            "c_vones": vones,
            "c_onesb": onesb,
            "c_eye": eye,
            "c_e2": e2,
            "c_den0": np.zeros((64, 512), np.float32),
        }
        if affine:
            gbs = np.stack([q_gamma, q_beta, k_gamma, k_beta])  # [4, 64]
            m["c_gb"] = np.ascontiguousarray(
                np.broadcast_to(gbs[None], (128, 4, HD)).astype(np.float32)
            )
        in_maps.append(m)
    return in_maps


_NC_CACHE = {}

LAST_RESULTS = None


def kernel(x, w_qkv, b_qkv, w_proj, b_proj, q_gamma, q_beta, k_gamma, k_beta,
           **unused):
    global LAST_RESULTS
    x = np.asarray(x, np.float32)
    w_qkv = np.asarray(w_qkv, np.float32)
    b_qkv = np.asarray(b_qkv, np.float32)
    w_proj = np.asarray(w_proj, np.float32)
    b_proj = np.asarray(b_proj, np.float32)
    q_gamma = np.asarray(q_gamma, np.float32)
    q_beta = np.asarray(q_beta, np.float32)
    k_gamma = np.asarray(k_gamma, np.float32)
    k_beta = np.asarray(k_beta, np.float32)

    B, S, _ = x.shape
    affine = not (
        np.all(q_gamma == 1) and np.all(k_gamma == 1)
        and np.all(q_beta == 0) and np.all(k_beta == 0)
    )
    key = (B, S, affine)
    if key not in _NC_CACHE:
        _NC_CACHE[key] = build_nc(B, S, affine)
    nc = _NC_CACHE[key]

    in_maps = make_in_maps(
        x, w_qkv, b_qkv, w_proj, q_gamma, q_beta, k_gamma, k_beta, affine
    )
    trace = bool(int(os.environ.get("BASS_KERNEL_TRACE", "0")))
    res = run_bass_kernel_spmd(
        nc, in_maps, core_ids=list(range(NCORES)), trace=trace
    )
    LAST_RESULTS = res
    acc = np.zeros((B * S, D), np.float32)
    for r in res.results:
        acc += r["outp"].astype(np.float32)
    acc += b_proj[None, :]
    return acc.reshape(B, S, D)


# revision 26
# speedup vs baseline: 1.0309x; 1.0309x over previous
"""Multi-head self-attention (B=2, S=2048, D=1024, H=16) on 8 TRN2 NeuronCores.

Tensor-parallel over heads: each core owns 2 heads. Accepts FULL inputs,
returns FULL output. Host pre-transposes x and slices per-head weights;
each core computes qkv -> per-head LayerNorm -> attention -> partial
output projection (over its 128 embed dims); host sums the 8 partials
and adds the projection bias.

v2 layout of the per-core program:
  passA: per 128-token block: qkv matmuls -> grouped bn_stats -> raw
         q/k + v evicted to SBUF (bf16).
  passB: LN constants for all 32 blocks batched in ~8 wide vector ops
         (combine even/odd bn_stats pipes, one ACT sqrt, one DVE
         approx-reciprocal).
  passC: per block: 4 fused tensor_scalar LN applies (bf16 4x mode),
         2 PE transposes, one copy into the [dim, token] q/k store.
  phase2: software-pipelined attention: scores run 2 k-blocks ahead of
         the exp stream, A@V runs 2 behind, the softmax denominator is
         inverted with a DVE approx reciprocal (no ACT table switches
         ever - ACT does only the exp stream), and each chunk's
         projection is deferred one chunk to fill the PE bubble at the
         chunk boundary.
"""

import os
import sys

import numpy as np

for _p in ("/opt/trn_rl_repo", "/root/.axon_site/_ro/trn_rl_repo"):
    if os.path.isdir(_p) and _p not in sys.path:
        sys.path.insert(0, _p)
        break

import concourse.bass as bass  # noqa: E402
import concourse.bacc as bacc  # noqa: E402
import concourse.tile as tile  # noqa: E402
from concourse import mybir  # noqa: E402
from concourse.bass_utils import run_bass_kernel_spmd  # noqa: E402

F32 = mybir.dt.float32
F32R = mybir.dt.float32r
BF16 = mybir.dt.bfloat16
AF = mybir.ActivationFunctionType
ALU = mybir.AluOpType

NCORES = 8
D = 1024
H = 16
HD = 64
HPC = H // NCORES          # heads per core = 2
DPC = HPC * HD             # embed dims per core = 128
KCH_H = D // 128           # contraction chunks (host-side constant)
EPS = 1e-5


def _r(ap):
    return ap.bitcast(F32R)


def build_nc(B, S, affine):
    """Build the SPMD Bass program for one core (same program, 8 cores)."""
    T = B * S                      # total token columns
    NTB = T // 128                 # 128-token blocks (32)
    NCH = T // 512                 # 512-token chunks (8)
    QC = S // 512                  # q-chunks per batch (4)
    KB = S // 128                  # k-blocks per batch (16)
    KCH = D // 128                 # contraction chunks (8)
    SCALE = 1.0 / np.sqrt(HD)

    nc = bacc.Bacc(
        "TRN2",
        target_bir_lowering=False,
        debug=False,
        enable_asserts=True,
        num_devices=NCORES,
    )

    xT = nc.dram_tensor("xT", [D, T], BF16, kind="ExternalInput").ap()
    wq = nc.dram_tensor(
        "wt_qkv", [128, KCH, 3 * DPC], BF16, kind="ExternalInput"
    ).ap()
    bq = nc.dram_tensor("b_qkv_s", [1, 3 * DPC], BF16, kind="ExternalInput").ap()
    wp = nc.dram_tensor("wt_proj", [DPC, D], BF16, kind="ExternalInput").ap()
    ones = nc.dram_tensor("c_ones", [1, 512], F32R, kind="ExternalInput").ap()
    vones = nc.dram_tensor(
        "c_vones", [128, HPC, NTB, 1], BF16, kind="ExternalInput"
    ).ap()
    onesb = nc.dram_tensor("c_onesb", [1, 512], BF16, kind="ExternalInput").ap()
    e2 = nc.dram_tensor("c_e2", [64, 128], F32R, kind="ExternalInput").ap()
    den0 = nc.dram_tensor("c_den0", [64, 512], F32R, kind="ExternalInput").ap()
    eye = nc.dram_tensor("c_eye", [128, 128], BF16, kind="ExternalInput").ap()
    if affine:
        gb = nc.dram_tensor("c_gb", [128, 4, HD], F32, kind="ExternalInput").ap()
    outp = nc.dram_tensor("outp", [T, D], BF16, kind="ExternalOutput").ap()

    from contextlib import ExitStack

    with tile.TileContext(nc) as tc, ExitStack() as stack:
        const = stack.enter_context(tc.tile_pool(name="const", bufs=1))
        persist = stack.enter_context(tc.tile_pool(name="persist", bufs=1))

        # weights needed by passA go out first on the sync queue; the
        # late-use constants ride the gpsimd (SWDGE) queue so the first
        # x chunk isn't stuck behind them.
        bq_sb = const.tile([1, 3 * DPC], BF16, tag="bq")
        nc.sync.dma_start(out=bq_sb, in_=bq)
        onesb_sb = const.tile([1, 512], BF16, tag="onesb")
        nc.sync.dma_start(out=onesb_sb, in_=onesb)
        wq_sb = const.tile([128, KCH, 3 * DPC], BF16, tag="wq")
        nc.sync.dma_start(out=wq_sb, in_=wq)
        eye_sb = const.tile([128, 128], BF16, tag="eye")
        nc.gpsimd.dma_start(out=eye_sb, in_=eye)
        ones_sb = const.tile([1, 512], F32R, tag="ones")
        nc.gpsimd.dma_start(out=ones_sb, in_=ones)
        e2_sb = const.tile([64, 128], F32R, tag="e2")
        nc.gpsimd.dma_start(out=e2_sb, in_=e2)
        wp_sb = const.tile([DPC, D], BF16, tag="wp")
        nc.gpsimd.dma_start(out=wp_sb, in_=wp)
        eps_sb = const.tile([128, 1], F32, tag="eps")
        nc.vector.memset(eps_sb, EPS)
        if affine:
            gb_sb = const.tile([128, 4, HD], F32, tag="gb")
            nc.gpsimd.dma_start(out=gb_sb, in_=gb)

        # persistent intermediates
        qkT = persist.tile([128, 2, T], BF16, tag="qkT")   # plane0=q^T plane1=k^T
        vO = persist.tile([128, HPC, NTB, HD + 1], BF16, tag="vO")
        aT = persist.tile([128, T], BF16, tag="aT")        # attention out^T
        den64 = [
            persist.tile([64, 512], F32R, tag=f"den64{i}", name=f"den64{i}")
            for i in range(2)
        ]
        nc.gpsimd.dma_start(out=den64[0], in_=den0)
        nc.gpsimd.dma_start(out=den64[1], in_=den0)
        nc.gpsimd.dma_start(out=vO[:, :, :, HD : HD + 1], in_=vones)

        # ------- Phase 1: per-block qkv + LN + transpose pipeline --------
        # Work spread over three engines so no single one binds:
        #   PE:  bias+qkv matmuls, 2 transposes
        #   DVE: bn_stats/aggr, rstd recip, -mu*rstd
        #   ACT: sqrt, the 4 LN applies (scale/bias APs), v cast, qkT evict
        with (
            tc.tile_pool(name="xt", bufs=2) as xt_pool,
            tc.tile_pool(name="qkv_ps", bufs=4, space="PSUM") as qkv_ps,
            tc.tile_pool(name="stats", bufs=4) as stats_pool,
            tc.tile_pool(name="qn", bufs=3) as qn_pool,
            tc.tile_pool(name="t_ps", bufs=3, space="PSUM") as t_ps,
        ):
            xTr = xT.rearrange("(c p) t -> p c t", p=128)
            xt0s = []
            for q in range(4):
                x0 = xt_pool.tile(
                    [128, KCH, 128], BF16, tag=f"xt0_{q}", name=f"xt0_{q}"
                )
                nc.sync.dma_start(
                    out=x0, in_=xTr[:, :, q * 128 : (q + 1) * 128]
                )
                xt0s.append(x0)
            for n in range(NCH):
                if n > 0:
                    xt = xt_pool.tile([128, KCH, 512], BF16, tag="xt")
                    nc.sync.dma_start(
                        out=xt, in_=xTr[:, :, n * 512 : (n + 1) * 512]
                    )
                for tbl in range(4):
                    tb = n * 4 + tbl
                    xts = xt0s[tbl] if n == 0 else xt[
                        :, :, tbl * 128 : (tbl + 1) * 128
                    ]
                    ps = qkv_ps.tile([128, 3 * DPC], F32, tag="ps")
                    nc.tensor.matmul(
                        ps,
                        lhsT=onesb_sb[0:1, 0:128],
                        rhs=bq_sb,
                        start=True,
                        stop=False,
                    )
                    for k in range(KCH):
                        nc.tensor.matmul(
                            ps,
                            lhsT=xts[:, k, :],
                            rhs=wq_sb[:, k, :],
                            start=False,
                            stop=(k == KCH - 1),
                        )
                    # host interleaves the two heads' channels in the q
                    # and k column blocks, so one bn_stats per block gives
                    # per-head stats via its even/odd split pipes:
                    # st = [cnt_e, mean_h0, M2_h0, cnt_o, mean_h1, M2_h1]
                    st = stats_pool.tile([128, 2, 6], F32, tag="st")
                    nc.vector.bn_stats(out=st[:, 0], in_=ps[:, 0:DPC])
                    nc.vector.bn_stats(out=st[:, 1], in_=ps[:, DPC : 2 * DPC])
                    stf = st.rearrange("p a (b t) -> p (a b) t", t=3)
                    sd = stats_pool.tile([128, 4], F32, tag="sd")
                    nc.scalar.activation(
                        out=sd, in_=stf[:, :, 2], func=AF.Sqrt,
                        bias=eps_sb, scale=1.0 / HD,
                    )
                    rstd = stats_pool.tile([128, 4], F32, tag="rstd")
                    nc.vector.reciprocal(out=rstd, in_=sd)
                    negmu = stats_pool.tile([128, 4], F32, tag="negmu")
                    nc.vector.tensor_scalar(
                        out=negmu, in0=stf[:, :, 1], scalar1=-1.0,
                        scalar2=None, op0=ALU.mult,
                    )
                    nmr = stats_pool.tile([128, 4], F32, tag="nmr")
                    nc.vector.tensor_mul(nmr, negmu, rstd)
                    qn = qn_pool.tile([128, 2, 2 * HD], BF16, tag="qn")
                    for g in range(4):
                        pl, hh = g // 2, g % 2
                        dsl = qn[:, pl, hh * HD : (hh + 1) * HD]
                        src_g = ps[:, pl * DPC : (pl + 1) * DPC].rearrange(
                            "p (c two) -> p two c", two=2
                        )[:, hh]
                        if pl == 0:
                            # q applies on ACT (idle in phase 1)
                            nc.scalar.activation(
                                out=dsl,
                                in_=src_g,
                                func=AF.Identity,
                                bias=nmr[:, g : g + 1],
                                scale=rstd[:, g : g + 1],
                            )
                        else:
                            # k applies on DVE
                            nc.vector.tensor_scalar(
                                out=dsl,
                                in0=src_g,
                                scalar1=stf[:, g : g + 1, 1],
                                scalar2=rstd[:, g : g + 1],
                                op0=ALU.subtract,
                                op1=ALU.mult,
                            )
                        if affine:
                            nc.vector.tensor_mul(dsl, dsl, gb_sb[:, 2 * pl, :])
                            nc.vector.tensor_add(dsl, dsl, gb_sb[:, 2 * pl + 1, :])
                    nc.scalar.copy(
                        out=vO[:, :, tb, 0:HD],
                        in_=ps[:, 2 * DPC :].rearrange("p (h d) -> p h d", d=HD),
                    )
                    ts_ = slice(tb * 128, (tb + 1) * 128)
                    tp = t_ps.tile([128, 256], BF16, tag="tp")
                    nc.tensor.transpose(tp[:, 0:128], qn[:, 0, :], eye_sb)
                    nc.tensor.transpose(tp[:, 128:256], qn[:, 1, :], eye_sb)
                    nc.scalar.copy(
                        out=qkT[:, :, ts_],
                        in_=tp.rearrange("p (c d) -> p c d", d=128),
                    )

        # ---------------- Phase 2: attention -----------------------------
        with (
            tc.tile_pool(name="sc_ps", bufs=2, space="PSUM") as sc_ps,
            tc.tile_pool(name="o_ps", bufs=1, space="PSUM") as o_ps,
            tc.tile_pool(name="sm_ps", bufs=2, space="PSUM") as sm_ps,
            tc.tile_pool(name="exps", bufs=4) as exps,
            tc.tile_pool(name="stage2", bufs=2) as stage2,
            tc.tile_pool(name="ostage", bufs=2) as ostage,
        ):
            seq = [(ci, kb) for ci in range(B * QC) for kb in range(KB)]
            ooms = {}
            exts = {}

            def emit_scores_exp(ci, kb):
                b, qc = divmod(ci, QC)
                cols = slice(b * S + qc * 512, b * S + (qc + 1) * 512)
                gkb = b * KB + kb
                ks = slice(gkb * 128, (gkb + 1) * 128)
                scp = sc_ps.tile([128, HPC, 512], F32, tag="s", name="scp")
                for h in range(HPC):
                    hp = slice(h * HD, (h + 1) * HD)
                    nc.tensor.matmul(
                        scp[:, h, :],
                        lhsT=qkT[hp, 1, ks],
                        rhs=qkT[hp, 0, cols],
                        start=True,
                        stop=True,
                    )
                ex = exps.tile([128, HPC, 512], BF16, tag="ex", name="ex")
                nc.scalar.activation(out=ex, in_=scp, func=AF.Exp, scale=SCALE)
                exts[(ci, kb)] = ex

            def emit_av(ci, kb):
                b, _ = divmod(ci, QC)
                gkb = b * KB + kb
                if kb == 0:
                    ooms[ci] = o_ps.tile(
                        [HD + 1, HPC, 512], F32, tag="o", name="oom"
                    )
                oom = ooms[ci]
                ex = exts.pop((ci, kb))
                for h in range(HPC):
                    nc.tensor.matmul(
                        oom[:, h, :],
                        lhsT=vO[:, h, gkb, :],
                        rhs=ex[:, h, :],
                        start=(kb == 0),
                        stop=(kb == KB - 1),
                    )

            def emit_tail(ci):
                b, qc = divmod(ci, QC)
                cols = slice(b * S + qc * 512, b * S + (qc + 1) * 512)
                oom = ooms.pop(ci)
                dn = den64[ci % 2]
                for h in range(HPC):
                    nc.vector.tensor_copy(
                        out=dn[32 * h : 32 * h + 1, :],
                        in_=oom[HD : HD + 1, h, :],
                    )
                rb = sm_ps.tile([128, 512], F32, tag="sm", name="rb")
                nc.tensor.matmul(rb, lhsT=e2_sb, rhs=dn, start=True, stop=True)
                if ci == B * QC - 1:
                    # cover the reciprocal+normalize window of the final
                    # drain with harmless matmuls so HAM stays warm
                    dm2 = sc_ps.tile([128, HPC, 512], F32, tag="s", name="dm2")
                    for i in range(4):
                        nc.tensor.matmul(
                            dm2[:, i % 2, :],
                            lhsT=e2_sb,
                            rhs=den64[(ci + 1) % 2],
                            start=True,
                            stop=True,
                        )
                rbs = stage2.tile([128, 512], F32, tag="rbs", name="rbs")
                nc.vector.reciprocal_approx_fast(out=rbs, in_=rb)
                for h in range(HPC):
                    nc.vector.tensor_mul(
                        aT[h * HD : (h + 1) * HD, cols],
                        oom[0:HD, h, :],
                        rbs[h * HD : (h + 1) * HD, :],
                    )

            def emit_proj(ci, tbls, alt=False):
                for tbl in tbls:
                    tb = ci * 4 + tbl
                    ob = ostage.tile([128, D], BF16, tag="ob")
                    for nn in range(D // 512):
                        pps = sm_ps.tile([128, 512], F32, tag="sm", name="pps")
                        nc.tensor.matmul(
                            pps,
                            lhsT=aT[:, tb * 128 : (tb + 1) * 128],
                            rhs=wp_sb[:, nn * 512 : (nn + 1) * 512],
                            start=True,
                            stop=True,
                        )
                        osl = ob[:, nn * 512 : (nn + 1) * 512]
                        if alt and (tbl + nn) % 2:
                            # in the post-exp drain ACT is free: share casts
                            nc.scalar.copy(out=osl, in_=pps)
                        else:
                            nc.vector.tensor_copy(out=osl, in_=pps)
                    nc.gpsimd.dma_start(
                        out=outp[tb * 128 : (tb + 1) * 128, :], in_=ob
                    )

            last = B * QC - 1
            for idx, (ci, kb) in enumerate(seq):
                emit_scores_exp(ci, kb)
                if idx >= 2:
                    emit_av(*seq[idx - 2])
                if kb == 1 and ci >= 1:
                    # proj of chunk ci-2 (data long ready) split around the
                    # tail of ci-1 so its matmuls cover the PE bubble while
                    # the DVE runs the denominator-reciprocal chain, but its
                    # PSUM casts don't delay that chain.
                    if ci >= 2:
                        emit_proj(ci - 2, (0, 1))
                    emit_tail(ci - 1)
                elif kb == 3 and ci >= 2:
                    emit_proj(ci - 2, (2, 3))
                elif ci == last and kb == 8:
                    # pull the second-to-last chunk's proj into this chunk
                    # so only proj(last) remains after the exp stream ends
                    emit_proj(last - 1, (0, 1))
                elif ci == last and kb == 12:
                    emit_proj(last - 1, (2, 3))
            emit_av(*seq[-2])
            emit_av(*seq[-1])
            # harmless matmuls keep the PE's HAM clock warm through the
            # serial reciprocal/projection drain (idle >3.4us would halve
            # the PE clock for the rest of the kernel)
            dmy = sc_ps.tile([128, HPC, 512], F32, tag="s", name="dmy")
            for i in range(6):
                nc.tensor.matmul(
                    dmy[:, i % 2, :],
                    lhsT=e2_sb,
                    rhs=den64[(last + 1) % 2],
                    start=True,
                    stop=True,
                )
            emit_tail(last)
            emit_proj(last, (0, 1, 2, 3), alt=True)

    nc.compile()
    return nc


def make_in_maps(x, w_qkv, b_qkv, w_proj, q_gamma, q_beta, k_gamma, k_beta,
                 affine):
    import ml_dtypes

    bf = ml_dtypes.bfloat16
    B, S, _ = x.shape
    T = B * S
    xT = np.ascontiguousarray(x.reshape(T, D).T).astype(bf)
    ones = np.ones((1, 512), np.float32)
    onesb = np.ones((1, 512), bf)
    vones = np.ones((128, HPC, (T // 128), 1), bf)
    eye = np.eye(128, dtype=np.float32).astype(bf)
    in_maps = []
    for c in range(NCORES):
        rs = slice(c * DPC, (c + 1) * DPC)
        w_slice = np.concatenate(
            [w_qkv[rs], w_qkv[D:2 * D][rs.start:rs.stop],
             w_qkv[2 * D:][rs.start:rs.stop]],
            axis=0,
        )  # [384, 1024]
        b_slice = np.concatenate(
            [b_qkv[rs], b_qkv[D:2 * D][rs.start:rs.stop],
             b_qkv[2 * D:][rs.start:rs.stop]]
        )[None, :]  # [1, 384]
        # interleave the two heads' channels inside the q and the k blocks
        # (h0 -> even, h1 -> odd) so bn_stats' even/odd pipes separate them
        perm = np.arange(3 * DPC)
        for blk in range(2):
            base = blk * DPC
            iv = np.empty(DPC, np.int64)
            iv[0::2] = base + np.arange(HD)
            iv[1::2] = base + HD + np.arange(HD)
            perm[base : base + DPC] = iv
        w_slice = w_slice[perm]
        b_slice = b_slice[:, perm]
        wq_l = np.ascontiguousarray(
            w_slice.T.reshape(KCH_H, 128, 3 * DPC).transpose(1, 0, 2)
        ).astype(bf)
        e2 = np.zeros((64, 128), np.float32)
        e2[0, 0:HD] = 1.0
        e2[32, HD:128] = 1.0
        m = {
            "xT": xT,
            "wt_qkv": wq_l,
            "b_qkv_s": np.ascontiguousarray(b_slice).astype(bf),
            "wt_proj": np.ascontiguousarray(w_proj[:, rs].T).astype(bf),
            "c_ones": ones,
            "c_vones": vones,
            "c_onesb": onesb,
            "c_eye": eye,
            "c_e2": e2,
            "c_den0": np.zeros((64, 512), np.float32),
        }
        if affine:
            gbs = np.stack([q_gamma, q_beta, k_gamma, k_beta])  # [4, 64]
            m["c_gb"] = np.ascontiguousarray(
                np.broadcast_to(gbs[None], (128, 4, HD)).astype(np.float32)
            )
        in_maps.append(m)
    return in_maps


_NC_CACHE = {}

LAST_RESULTS = None


def kernel(x, w_qkv, b_qkv, w_proj, b_proj, q_gamma, q_beta, k_gamma, k_beta,
           **unused):
    global LAST_RESULTS
    x = np.asarray(x, np.float32)
    w_qkv = np.asarray(w_qkv, np.float32)
    b_qkv = np.asarray(b_qkv, np.float32)
    w_proj = np.asarray(w_proj, np.float32)
    b_proj = np.asarray(b_proj, np.float32)
    q_gamma = np.asarray(q_gamma, np.float32)
    q_beta = np.asarray(q_beta, np.float32)
    k_gamma = np.asarray(k_gamma, np.float32)
    k_beta = np.asarray(k_beta, np.float32)

    B, S, _ = x.shape
    affine = not (
        np.all(q_gamma == 1) and np.all(k_gamma == 1)
        and np.all(q_beta == 0) and np.all(k_beta == 0)
    )
    key = (B, S, affine)
    if key not in _NC_CACHE:
        _NC_CACHE[key] = build_nc(B, S, affine)
    nc = _NC_CACHE[key]

    in_maps = make_in_maps(
        x, w_qkv, b_qkv, w_proj, q_gamma, q_beta, k_gamma, k_beta, affine
    )
    trace = bool(int(os.environ.get("BASS_KERNEL_TRACE", "0")))
    res = run_bass_kernel_spmd(
        nc, in_maps, core_ids=list(range(NCORES)), trace=trace
    )
    LAST_RESULTS = res
    acc = np.zeros((B * S, D), np.float32)
    for r in res.results:
        acc += r["outp"].astype(np.float32)
    acc += b_proj[None, :]
    return acc.reshape(B, S, D)
